# revision 66
# baseline (speedup 1.0000x reference)
"""Multi-head attention (B=2, L=2048, D=1024, H=16, RoPE) on 8 TRN2 NeuronCores.

Sharding: 32 (batch, head) pairs / 8 cores -> core c handles batch c//4 and
heads 4*(c%4) .. 4*(c%4)+3. QKV / out projections are column/row split per
head group; the inter-head-group sum of out-projection partials (and the bout
bias) is applied on the host during unshard.

Per-core dataflow (all matmuls in float32r = full-rate fp32 mode):
  - host feeds xT = x[b].T  [D, L]  (contraction dim on partitions)
  - qT,kT in [feat, L] layout: matmul(lhsT=Wqk_tile, rhs=xT_tile); bias is
    seeded into PSUM via a K=1 matmul; RoPE applied on DVE during evacuation
    with host cos/sin tables (sin signed + input-partition-indexed)
  - v in [L, feat] layout: matmul(lhsT=xT_tile, rhs=Wv_tile) into [128, 4*65]
    stationary tiles with a ones column per head (PV row 64 = softmax denom)
  - S^T tile = matmul(lhsT=kT[64,128], rhs=qT[64,512]): keys on partitions;
    exp on ScalarE batched over [128,1024] PSUM spans (amortize the +352cyc
    ACT op overhead); PV = matmul(lhsT=v[128,65], rhs=E[128,512-chunk])
    accumulated over 16 key tiles into a [65, 2048] PSUM accumulator
  - normalize: DVE reciprocal of row 64, bounce through DRAM with a
    partition-broadcast DMA, one [64,2048] DVE multiply -> OT stack
  - out-proj: matmul(lhsT=OT_stack[feat, L-chunk], rhs=Wout_rows)
"""
import sys
import numpy as np
import ml_dtypes

try:
    import concourse.bass as bass  # noqa: F401
except ImportError:
    sys.path.insert(0, "/opt/trn_rl_repo")

import concourse.bass as bass
import concourse.mybir as mybir
import concourse.tile as tile
from concourse import bacc
from concourse.bass_utils import run_bass_kernel_spmd

B, L, D = 2, 2048, 1024
H = 16                     # total heads
HPC = 4                    # heads per core
HD = 64                    # head dim
N_CORES = 8
ROPE_BASE = 10000.0

F32 = mybir.dt.float32
F32R = mybir.dt.float32r

LC = 512                   # matmul moving-dim chunk
NLC = L // LC              # 4
NLT = L // 128             # 16 L tiles
NDT = D // 128             # 8 contraction tiles for projections
QK = 2 * HPC * HD          # 512 rows of q+k features
NMT = QK // 128            # 4 m-tiles (0,1 = q heads 0-3; 2,3 = k heads 0-3)
VF = HPC * HD              # 256 v features


def _build_nc():
    nc = bacc.Bacc("TRN2", target_bir_lowering=False, debug=False,
                   num_devices=N_CORES)

    xT_e = nc.declare_dram_parameter("xT", [D, L], mybir.dt.bfloat16, isOutput=False)
    wqk_e = nc.declare_dram_parameter("wqk", [D, QK], mybir.dt.bfloat16, isOutput=False)
    wv_e = nc.declare_dram_parameter("wv", [D, VF], mybir.dt.bfloat16, isOutput=False)
    wout_e = nc.declare_dram_parameter("wout", [VF, D], F32R, isOutput=False)
    cos2_e = nc.declare_dram_parameter("cos2", [128, L], mybir.dt.bfloat16, isOutput=False)
    sin2_e = nc.declare_dram_parameter("sin2", [128, L], mybir.dt.bfloat16, isOutput=False)
    bqk_e = nc.declare_dram_parameter("bqk", [128, NMT], F32, isOutput=False)
    bv_e = nc.declare_dram_parameter("bv", [1, VF], F32R, isOutput=False)
    ones_e = nc.declare_dram_parameter("ones", [1, LC], F32R, isOutput=False)
    vones_e = nc.declare_dram_parameter("vones", [128, HPC], F32R, isOutput=False)
    out_e = nc.declare_dram_parameter("out", [L, D], F32, isOutput=True)

    with tile.TileContext(nc) as tc:
        import contextlib
        with contextlib.ExitStack() as stack:
            persist = stack.enter_context(tc.tile_pool(name="persist", bufs=1))
            dram = stack.enter_context(
                tc.tile_pool(name="dram", bufs=2, space="DRAM"))

            # ---- persistent tiles ------------------------------------------
            qkT = [persist.tile([128, L], mybir.dt.bfloat16, tag=f"qkT{i}", name=f"qkT{i}")
                   for i in range(NMT)]
            v_sb = [persist.tile([128, HPC * (HD + 1)], F32R, tag=f"v{i}",
                                 name=f"v{i}") for i in range(NLT)]
            otT = [persist.tile([128, L], F32R, tag=f"otT{i}", name=f"otT{i}")
                   for i in range(2)]
            wout_sb = [persist.tile([128, D], F32R, tag=f"wout{i}",
                                    name=f"wout{i}") for i in range(2)]
            cos2 = persist.tile([128, L], mybir.dt.bfloat16, tag="cos2")
            sin2 = persist.tile([128, L], mybir.dt.bfloat16, tag="sin2")
            bqk_sb = persist.tile([128, NMT], F32, tag="bqk")
            bv_sb = persist.tile([1, VF], F32R, tag="bv")
            ones_sb = persist.tile([1, LC], F32R, tag="ones")


            # ---- phase A: projections (x and W tiles live only here) -------
            with tc.tile_pool(name="proj", bufs=1) as proj, \
                 tc.tile_pool(name="qkpsum", bufs=4, space="PSUM") as qkpsum, \
                 tc.tile_pool(name="vpsum", bufs=4, space="PSUM") as vpsum, \
                 tc.tile_pool(name="ptmp", bufs=3) as ptmp:
                xT_sb = [proj.tile([128, L], mybir.dt.bfloat16, tag=f"xT{i}", name=f"xT{i}")
                         for i in range(NDT)]
                wqk_sb = [proj.tile([128, QK], mybir.dt.bfloat16, tag=f"wqk{i}",
                                    name=f"wqk{i}") for i in range(NDT)]
                wv_sb = [proj.tile([128, VF], mybir.dt.bfloat16, tag=f"wv{i}",
                                   name=f"wv{i}") for i in range(NDT)]
                # inputs first, constants after
                for i in range(NDT):
                    nc.sync.dma_start(out=xT_sb[i], in_=xT_e[i * 128:(i + 1) * 128, :])
                    nc.sync.dma_start(out=wqk_sb[i], in_=wqk_e[i * 128:(i + 1) * 128, :])
                    nc.sync.dma_start(out=wv_sb[i], in_=wv_e[i * 128:(i + 1) * 128, :])
                nc.sync.dma_start(out=cos2, in_=cos2_e[:, :])
                nc.sync.dma_start(out=sin2, in_=sin2_e[:, :])
                nc.sync.dma_start(out=bqk_sb, in_=bqk_e[:, :])
                nc.sync.dma_start(out=bv_sb, in_=bv_e[:, :])
                nc.sync.dma_start(out=ones_sb, in_=ones_e[:, :])
                nc.sync.dma_start(out=wout_sb[0], in_=wout_e[0:128, :])
                nc.sync.dma_start(out=wout_sb[1], in_=wout_e[128:256, :])
                # ones column of each v stationary tile (col 64 per head)
                for lt in range(NLT):
                    nc.sync.dma_start(
                        out=v_sb[lt].rearrange("p (h e) -> p h e", h=HPC)[:, :, HD:HD + 1],
                        in_=vones_e.rearrange("p (h o) -> p h o", o=1))

                # qkT projection: stationary-major loop (amortize f32r LDW).
                # Order: pair-0 q/k first, then v (PV needs it before pair-1
                # S results matter), then pair-1 q/k.
                def project_qk(mt):
                    pss = [qkpsum.tile([128, LC], F32, tag="qkps",
                                       name=f"qkps{mt}_{lc}") for lc in range(NLC)]
                    for dt_ in range(NDT):
                        for lc in range(NLC):
                            nc.tensor.matmul(
                                pss[lc],
                                wqk_sb[dt_][:, mt * 128:(mt + 1) * 128],
                                xT_sb[dt_][:, lc * LC:(lc + 1) * LC],
                                start=(dt_ == 0), stop=(dt_ == NDT - 1))
                    # RoPE evacuation: ACT (idle here) copies PSUM->bf16 SBUF,
                    # then all DVE ops run in bf16 2x mode.
                    for lc in range(NLC):
                        cs = slice(lc * LC, (lc + 1) * LC)
                        ps = pss[lc]
                        t0 = ptmp.tile([128, LC], mybir.dt.bfloat16, tag="t0")
                        # PSUM->bf16 SBUF with the per-feature qk bias fused
                        nc.scalar.activation(
                            out=t0, in_=ps,
                            func=mybir.ActivationFunctionType.Identity,
                            bias=bqk_sb[:, mt:mt + 1], scale=1.0)
                        ta = ptmp.tile([128, LC], mybir.dt.bfloat16, tag="ta")
                        nc.vector.tensor_mul(ta, t0, cos2[:, cs])
                        tb = ptmp.tile([128, LC], mybir.dt.bfloat16, tag="tb")
                        # rotate_half: out block o0 reads input block i0=o0^32;
                        # sin2 is indexed by the INPUT block (host-prearranged)
                        for blk in range(4):
                            o0 = blk * 32
                            i0 = (blk ^ 1) * 32
                            nc.vector.tensor_mul(
                                tb[o0:o0 + 32, :], t0[i0:i0 + 32, :],
                                sin2[i0:i0 + 32, cs])
                        nc.vector.tensor_add(qkT[mt][:, cs], ta, tb)

                def project_v(lt):
                    ps = vpsum.tile([128, VF], F32, tag="vps", name=f"vps{lt}")
                    nc.tensor.matmul(ps, ones_sb[:, 0:128], bv_sb,
                                     start=True, stop=False)
                    for dt_ in range(NDT):
                        nc.tensor.matmul(
                            ps,
                            xT_sb[dt_][:, lt * 128:(lt + 1) * 128],
                            wv_sb[dt_],
                            start=False, stop=(dt_ == NDT - 1))
                    # evacuate on ScalarE: idle in this phase, and it frees
                    # the PSUM slot without queuing behind the DVE RoPE ops
                    nc.scalar.copy(
                        out=v_sb[lt].rearrange("p (h e) -> p h e", h=HPC)[:, :, 0:HD],
                        in_=ps.rearrange("p (h e) -> p h e", h=HPC))

                project_qk(0)
                project_qk(2)
                for lt in range(NLT):
                    project_v(lt)
                project_qk(1)
                project_qk(3)

            # ---- phase B: attention + interleaved out-projection -----------
            # Heads processed in PAIRS: both heads' S^T for one q-chunk land
            # in ONE [128,1024] PSUM tile (disjoint PE row groups via
            # tile_position), one exp covers both. The out-projection for a
            # head pair runs as soon as its otT columns are normalized,
            # writing a separate output partial per pair (host sums them).
            with tc.tile_pool(name="e_pool", bufs=9) as e_pool, \
                 tc.tile_pool(name="spsum", bufs=3, space="PSUM") as spsum, \
                 tc.tile_pool(name="opsum", bufs=2, space="PSUM") as opsum, \
                 tc.tile_pool(name="btmp", bufs=2) as btmp:
                def normalize(ot_sb, h, qc):
                    # rows 0..63 scaled by 1/row64; ot_sb is an SBUF copy so
                    # the PSUM slot is already released. DVE reciprocal costs
                    # ~6cyc per FREE element, so reshape the [1,512] row to
                    # [128,4] via a DRAM bounce, recip, gather back, and
                    # partition-broadcast to [64,512].
                    prow = (h % 2) * HD
                    rdram = dram.tile([1, LC], F32, tag="rdram",
                                      name=f"rd{h}_{qc}")
                    nc.sync.dma_start(out=rdram, in_=ot_sb[HD:HD + 1, :])
                    rsq = btmp.tile([128, LC // 128], F32, tag="rsq",
                                    name=f"rsq{h}_{qc}")
                    nc.sync.dma_start(
                        out=rsq,
                        in_=rdram.rearrange("o (p f) -> (o p) f", p=128))
                    rrec = btmp.tile([128, LC // 128], F32, tag="rrec",
                                     name=f"rrec{h}_{qc}")
                    nc.vector.reciprocal(out=rrec, in_=rsq)
                    rdram2 = dram.tile([1, LC], F32, tag="rdram2",
                                       name=f"rd2{h}_{qc}")
                    nc.sync.dma_start(
                        out=rdram2.rearrange("o (p f) -> (o p) f", p=128),
                        in_=rrec)
                    bc_sb = btmp.tile([HD, LC], F32, tag="bcsb",
                                      name=f"bc{h}_{qc}")
                    bcast_src = bass.AP(
                        tensor=rdram2.tensor, offset=rdram2.offset,
                        ap=[[0, HD]] + list(rdram2.ap[1:]))
                    nc.sync.dma_start(out=bc_sb, in_=bcast_src)
                    nc.vector.tensor_mul(
                        otT[h // 2][prow:prow + HD, qc * LC:(qc + 1) * LC],
                        ot_sb[0:HD, :], bc_sb)

                for hp in range(2):
                    qt = qkT[hp]
                    kt_t = qkT[2 + hp]
                    ha, hb = 2 * hp, 2 * hp + 1
                    vca = slice(ha * (HD + 1), (ha + 1) * (HD + 1))
                    vcb = slice(hb * (HD + 1), (hb + 1) * (HD + 1))
                    for qc in range(NLC):
                        qs = slice(qc * LC, (qc + 1) * LC)
                        ot_a = opsum.tile([HD + 1, LC], F32, tag="otps",
                                          name=f"ota{hp}_{qc}")
                        ot_b = opsum.tile([HD + 1, LC], F32, tag="otps",
                                          name=f"otb{hp}_{qc}")
                        for kt in range(NLT):
                            ks = slice(kt * 128, (kt + 1) * 128)
                            # both heads' S^T into ONE tile: cols 0:512 =
                            # head a, 512:1024 = head b (disjoint row groups)
                            st = spsum.tile([128, 2 * LC], F32, tag="stps",
                                            name=f"st{hp}_{kt}_{qc}")
                            nc.tensor.matmul(
                                st[:, 0:LC],
                                kt_t[0:HD, ks], qt[0:HD, qs],
                                start=True, stop=True,
                                tile_position=(0, 0))
                            nc.tensor.matmul(
                                st[:, LC:2 * LC],
                                kt_t[HD:128, ks], qt[HD:128, qs],
                                start=True, stop=True,
                                tile_position=(HD, 0))
                            e_t = e_pool.tile([128, 2 * LC], F32R, tag="e",
                                              name=f"e{hp}_{kt}_{qc}")
                            nc.scalar.activation(
                                out=e_t, in_=st,
                                func=mybir.ActivationFunctionType.Exp,
                                scale=float(HD) ** -0.5)
                            nc.tensor.matmul(
                                ot_a, v_sb[kt][:, vca], e_t[:, 0:LC],
                                start=(kt == 0), stop=(kt == NLT - 1))
                            nc.tensor.matmul(
                                ot_b, v_sb[kt][:, vcb], e_t[:, LC:2 * LC],
                                start=(kt == 0), stop=(kt == NLT - 1))
                        ota_sb = btmp.tile([HD + 1, LC], F32, tag="otsb",
                                           bufs=6, name=f"otsa{hp}_{qc}")
                        nc.vector.tensor_copy(out=ota_sb, in_=ot_a)
                        otb_sb = btmp.tile([HD + 1, LC], F32, tag="otsb",
                                           bufs=6, name=f"otsb{hp}_{qc}")
                        nc.vector.tensor_copy(out=otb_sb, in_=ot_b)
                        normalize(ota_sb, ha, qc)
                        normalize(otb_sb, hb, qc)

            # ---- phase C: out-projection partial ---------------------------
            with tc.tile_pool(name="ypsum", bufs=4, space="PSUM") as ypsum, \
                 tc.tile_pool(name="ytmp", bufs=4) as ytmp:
                for lt in range(NLT):
                    for nch in range(2):
                        yps = ypsum.tile([128, LC], F32, tag="yps",
                                         name=f"yps{lt}_{nch}")
                        for ft in range(2):
                            nc.tensor.matmul(
                                yps,
                                otT[ft][:, lt * 128:(lt + 1) * 128],
                                wout_sb[ft][:, nch * LC:(nch + 1) * LC],
                                start=(ft == 0), stop=(ft == 1))
                        y_sb = ytmp.tile([128, LC], F32, tag="ysb",
                                         name=f"ysb{lt}_{nch}")
                        if (lt + nch) % 2 == 0:
                            nc.vector.tensor_copy(out=y_sb, in_=yps)
                        else:
                            nc.scalar.copy(out=y_sb, in_=yps)
                        nc.sync.dma_start(
                            out=out_e[lt * 128:(lt + 1) * 128,
                                      nch * LC:(nch + 1) * LC],
                            in_=y_sb)

    nc.compile()
    return nc


def _rope_tables():
    inv_freq = 1.0 / (ROPE_BASE ** (np.arange(0, HD, 2, dtype=np.float32) / HD))
    t = np.arange(L, dtype=np.float32)
    freqs = np.einsum("i,j->ij", t, inv_freq)            # [L, 32]
    emb = np.concatenate((freqs, freqs), axis=-1)        # [L, 64]
    cosT = np.cos(emb).T.astype(np.float32)              # [64, L]
    sinT = np.sin(emb).T.astype(np.float32)              # [64, L]
    # sin table is indexed by the INPUT partition of the rotate_half term:
    # out[0:32] reads in[32:64] -> table rows 32:64 hold -sin;
    # out[32:64] reads in[0:32] -> table rows 0:32 hold +sin.
    cos2 = np.concatenate([cosT, cosT], axis=0)          # [128, L]
    sin_signed = np.concatenate([sinT[:32], -sinT[32:]], axis=0)  # [64, L]
    sin2 = np.concatenate([sin_signed, sin_signed], axis=0)       # [128, L]
    return np.ascontiguousarray(cos2), np.ascontiguousarray(sin2)


_NC = None
TRACE = False          # test harness sets True to collect exec_time_ns
LAST_RESULT = None


def kernel(x, Wqkv, bqkv, Wout, bout):
    global _NC, LAST_RESULT
    if _NC is None:
        _NC = _build_nc()

    x = np.asarray(x, dtype=np.float32)
    Wqkv = np.asarray(Wqkv, dtype=np.float32)
    bqkv = np.asarray(bqkv, dtype=np.float32)
    Wout = np.asarray(Wout, dtype=np.float32)
    bout = np.asarray(bout, dtype=np.float32)

    cos2, sin2 = _rope_tables()

    in_maps = []
    for c in range(N_CORES):
        b = c // 4
        heads = [4 * (c % 4) + i for i in range(HPC)]
        xT = np.ascontiguousarray(x[b].T)                            # [D, L]
        q_cols = [Wqkv[:, h * HD:(h + 1) * HD] for h in heads]
        k_cols = [Wqkv[:, D + h * HD:D + (h + 1) * HD] for h in heads]
        v_cols = [Wqkv[:, 2 * D + h * HD:2 * D + (h + 1) * HD] for h in heads]
        wqk = np.ascontiguousarray(np.concatenate(q_cols + k_cols, axis=1))
        wv = np.ascontiguousarray(np.concatenate(v_cols, axis=1))
        bq = np.concatenate([bqkv[h * HD:(h + 1) * HD] for h in heads])
        bk = np.concatenate([bqkv[D + h * HD:D + (h + 1) * HD] for h in heads])
        bv = np.concatenate([bqkv[2 * D + h * HD:2 * D + (h + 1) * HD]
                             for h in heads])
        wout = np.ascontiguousarray(
            np.concatenate([Wout[h * HD:(h + 1) * HD, :] for h in heads],
                           axis=0))
        in_maps.append({
            "xT": xT.astype(ml_dtypes.bfloat16),
            "wqk": wqk.astype(ml_dtypes.bfloat16),
            "wv": wv.astype(ml_dtypes.bfloat16),
            "wout": wout,
            "cos2": cos2.astype(ml_dtypes.bfloat16),
            "sin2": sin2.astype(ml_dtypes.bfloat16),
            "bqk": np.ascontiguousarray(
                np.concatenate([bq, bk]).reshape(NMT, 128).T),
            "bv": np.ascontiguousarray(bv[None, :]),
            "ones": np.ones((1, LC), dtype=np.float32),
            "vones": np.ones((128, HPC), dtype=np.float32),
        })

    res = run_bass_kernel_spmd(_NC, in_maps, core_ids=list(range(N_CORES)),
                               trace=TRACE)
    LAST_RESULT = res

    out = np.zeros((B, L, D), dtype=np.float32)
    for c in range(N_CORES):
        out[c // 4] += res.results[c]["out"]
    out += bout[None, None, :]
    return out


# revision 67
# speedup vs baseline: 1.0172x; 1.0172x over previous
"""Multi-head attention (B=2, L=2048, D=1024, H=16, RoPE) on 8 TRN2 NeuronCores.

Sharding: 32 (batch, head) pairs / 8 cores -> core c handles batch c//4 and
heads 4*(c%4) .. 4*(c%4)+3. QKV / out projections are column/row split per
head group; the inter-head-group sum of out-projection partials (and the bout
bias) is applied on the host during unshard.

Per-core dataflow (all matmuls in float32r = full-rate fp32 mode):
  - host feeds xT = x[b].T  [D, L]  (contraction dim on partitions)
  - qT,kT in [feat, L] layout: matmul(lhsT=Wqk_tile, rhs=xT_tile); bias is
    seeded into PSUM via a K=1 matmul; RoPE applied on DVE during evacuation
    with host cos/sin tables (sin signed + input-partition-indexed)
  - v in [L, feat] layout: matmul(lhsT=xT_tile, rhs=Wv_tile) into [128, 4*65]
    stationary tiles with a ones column per head (PV row 64 = softmax denom)
  - S^T tile = matmul(lhsT=kT[64,128], rhs=qT[64,512]): keys on partitions;
    exp on ScalarE batched over [128,1024] PSUM spans (amortize the +352cyc
    ACT op overhead); PV = matmul(lhsT=v[128,65], rhs=E[128,512-chunk])
    accumulated over 16 key tiles into a [65, 2048] PSUM accumulator
  - normalize: DVE reciprocal of row 64, bounce through DRAM with a
    partition-broadcast DMA, one [64,2048] DVE multiply -> OT stack
  - out-proj: matmul(lhsT=OT_stack[feat, L-chunk], rhs=Wout_rows)
"""
import sys
import numpy as np
import ml_dtypes

try:
    import concourse.bass as bass  # noqa: F401
except ImportError:
    sys.path.insert(0, "/opt/trn_rl_repo")

import concourse.bass as bass
import concourse.mybir as mybir
import concourse.tile as tile
from concourse import bacc
from concourse.bass_utils import run_bass_kernel_spmd

B, L, D = 2, 2048, 1024
H = 16                     # total heads
HPC = 4                    # heads per core
HD = 64                    # head dim
N_CORES = 8
ROPE_BASE = 10000.0

F32 = mybir.dt.float32
F32R = mybir.dt.float32r

LC = 512                   # matmul moving-dim chunk
NLC = L // LC              # 4
NLT = L // 128             # 16 L tiles
NDT = D // 128             # 8 contraction tiles for projections
QK = 2 * HPC * HD          # 512 rows of q+k features
NMT = QK // 128            # 4 m-tiles (0,1 = q heads 0-3; 2,3 = k heads 0-3)
VF = HPC * HD              # 256 v features


def _build_nc():
    nc = bacc.Bacc("TRN2", target_bir_lowering=False, debug=False,
                   num_devices=N_CORES)

    xT_e = nc.declare_dram_parameter("xT", [D, L], mybir.dt.bfloat16, isOutput=False)
    wqk_e = nc.declare_dram_parameter("wqk", [D, QK], mybir.dt.bfloat16, isOutput=False)
    wv_e = nc.declare_dram_parameter("wv", [D, VF], mybir.dt.bfloat16, isOutput=False)
    wout_e = nc.declare_dram_parameter("wout", [VF, D], F32R, isOutput=False)
    cos2_e = nc.declare_dram_parameter("cos2", [128, L], mybir.dt.bfloat16, isOutput=False)
    sin2_e = nc.declare_dram_parameter("sin2", [128, L], mybir.dt.bfloat16, isOutput=False)
    bqk_e = nc.declare_dram_parameter("bqk", [128, NMT], F32, isOutput=False)
    bv_e = nc.declare_dram_parameter("bv", [1, VF], F32R, isOutput=False)
    ones_e = nc.declare_dram_parameter("ones", [1, LC], F32R, isOutput=False)
    vones_e = nc.declare_dram_parameter("vones", [128, HPC], F32R, isOutput=False)
    out_e = nc.declare_dram_parameter("out", [L, D], F32, isOutput=True)

    with tile.TileContext(nc) as tc:
        import contextlib
        with contextlib.ExitStack() as stack:
            persist = stack.enter_context(tc.tile_pool(name="persist", bufs=1))
            dram = stack.enter_context(
                tc.tile_pool(name="dram", bufs=2, space="DRAM"))

            # ---- persistent tiles ------------------------------------------
            qkT = [persist.tile([128, L], mybir.dt.bfloat16, tag=f"qkT{i}", name=f"qkT{i}")
                   for i in range(NMT)]
            v_sb = [persist.tile([128, HPC * (HD + 1)], F32R, tag=f"v{i}",
                                 name=f"v{i}") for i in range(NLT)]
            otT = [persist.tile([128, L], F32R, tag=f"otT{i}", name=f"otT{i}")
                   for i in range(2)]
            wout_sb = [persist.tile([128, D], F32R, tag=f"wout{i}",
                                    name=f"wout{i}") for i in range(2)]
            cos2 = persist.tile([128, L], mybir.dt.bfloat16, tag="cos2")
            sin2 = persist.tile([128, L], mybir.dt.bfloat16, tag="sin2")
            bqk_sb = persist.tile([128, NMT], F32, tag="bqk")
            bv_sb = persist.tile([1, VF], F32R, tag="bv")
            ones_sb = persist.tile([1, LC], F32R, tag="ones")


            # ---- phase A: projections (x and W tiles live only here) -------
            with tc.tile_pool(name="proj", bufs=1) as proj, \
                 tc.tile_pool(name="qkpsum", bufs=4, space="PSUM") as qkpsum, \
                 tc.tile_pool(name="vpsum", bufs=4, space="PSUM") as vpsum, \
                 tc.tile_pool(name="ptmp", bufs=3) as ptmp:
                xT_sb = [proj.tile([128, L], mybir.dt.bfloat16, tag=f"xT{i}", name=f"xT{i}")
                         for i in range(NDT)]
                wqk_sb = [proj.tile([128, QK], mybir.dt.bfloat16, tag=f"wqk{i}",
                                    name=f"wqk{i}") for i in range(NDT)]
                wv_sb = [proj.tile([128, VF], mybir.dt.bfloat16, tag=f"wv{i}",
                                   name=f"wv{i}") for i in range(NDT)]
                # inputs first, constants after
                for i in range(NDT):
                    nc.sync.dma_start(out=xT_sb[i], in_=xT_e[i * 128:(i + 1) * 128, :])
                    nc.sync.dma_start(out=wqk_sb[i], in_=wqk_e[i * 128:(i + 1) * 128, :])
                    nc.sync.dma_start(out=wv_sb[i], in_=wv_e[i * 128:(i + 1) * 128, :])
                nc.sync.dma_start(out=cos2, in_=cos2_e[:, :])
                nc.sync.dma_start(out=sin2, in_=sin2_e[:, :])
                nc.sync.dma_start(out=bqk_sb, in_=bqk_e[:, :])
                nc.sync.dma_start(out=bv_sb, in_=bv_e[:, :])
                nc.sync.dma_start(out=ones_sb, in_=ones_e[:, :])
                nc.sync.dma_start(out=wout_sb[0], in_=wout_e[0:128, :])
                nc.sync.dma_start(out=wout_sb[1], in_=wout_e[128:256, :])
                # ones column of each v stationary tile (col 64 per head)
                for lt in range(NLT):
                    nc.sync.dma_start(
                        out=v_sb[lt].rearrange("p (h e) -> p h e", h=HPC)[:, :, HD:HD + 1],
                        in_=vones_e.rearrange("p (h o) -> p h o", o=1))

                # qkT projection: stationary-major loop (amortize f32r LDW).
                # Order: pair-0 q/k first, then v (PV needs it before pair-1
                # S results matter), then pair-1 q/k.
                def project_qk(mt):
                    pss = [qkpsum.tile([128, LC], F32, tag="qkps",
                                       name=f"qkps{mt}_{lc}") for lc in range(NLC)]
                    for dt_ in range(NDT):
                        for lc in range(NLC):
                            nc.tensor.matmul(
                                pss[lc],
                                wqk_sb[dt_][:, mt * 128:(mt + 1) * 128],
                                xT_sb[dt_][:, lc * LC:(lc + 1) * LC],
                                start=(dt_ == 0), stop=(dt_ == NDT - 1))
                    # RoPE evacuation: ACT (idle here) copies PSUM->bf16 SBUF,
                    # then all DVE ops run in bf16 2x mode.
                    for lc in range(NLC):
                        cs = slice(lc * LC, (lc + 1) * LC)
                        ps = pss[lc]
                        t0 = ptmp.tile([128, LC], mybir.dt.bfloat16, tag="t0")
                        # PSUM->bf16 SBUF with the per-feature qk bias fused
                        nc.scalar.activation(
                            out=t0, in_=ps,
                            func=mybir.ActivationFunctionType.Identity,
                            bias=bqk_sb[:, mt:mt + 1], scale=1.0)
                        ta = ptmp.tile([128, LC], mybir.dt.bfloat16, tag="ta")
                        nc.vector.tensor_mul(ta, t0, cos2[:, cs])
                        tb = ptmp.tile([128, LC], mybir.dt.bfloat16, tag="tb")
                        # rotate_half: out block o0 reads input block i0=o0^32;
                        # sin2 is indexed by the INPUT block (host-prearranged)
                        for blk in range(4):
                            o0 = blk * 32
                            i0 = (blk ^ 1) * 32
                            nc.vector.tensor_mul(
                                tb[o0:o0 + 32, :], t0[i0:i0 + 32, :],
                                sin2[i0:i0 + 32, cs])
                        nc.vector.tensor_add(qkT[mt][:, cs], ta, tb)

                def project_v(lt):
                    ps = vpsum.tile([128, VF], F32, tag="vps", name=f"vps{lt}")
                    nc.tensor.matmul(ps, ones_sb[:, 0:128], bv_sb,
                                     start=True, stop=False)
                    for dt_ in range(NDT):
                        nc.tensor.matmul(
                            ps,
                            xT_sb[dt_][:, lt * 128:(lt + 1) * 128],
                            wv_sb[dt_],
                            start=False, stop=(dt_ == NDT - 1))
                    # evacuate on ScalarE: idle in this phase, and it frees
                    # the PSUM slot without queuing behind the DVE RoPE ops
                    nc.scalar.copy(
                        out=v_sb[lt].rearrange("p (h e) -> p h e", h=HPC)[:, :, 0:HD],
                        in_=ps.rearrange("p (h e) -> p h e", h=HPC))

                project_qk(0)
                project_qk(2)
                for lt in range(NLT):
                    project_v(lt)
                project_qk(1)
                project_qk(3)

            # ---- phase B: attention + interleaved out-projection -----------
            # Heads processed in PAIRS: both heads' S^T for one q-chunk land
            # in ONE [128,1024] PSUM tile (disjoint PE row groups via
            # tile_position), one exp covers both. The out-projection for a
            # head pair runs as soon as its otT columns are normalized,
            # writing a separate output partial per pair (host sums them).
            with tc.tile_pool(name="e_pool", bufs=8) as e_pool, \
                 tc.tile_pool(name="spsum", bufs=3, space="PSUM") as spsum, \
                 tc.tile_pool(name="opsum", bufs=2, space="PSUM") as opsum, \
                 tc.tile_pool(name="btmp", bufs=2) as btmp:
                def normalize(ot_sb, h, qc):
                    # rows 0..63 scaled by 1/row64; ot_sb is an SBUF copy so
                    # the PSUM slot is already released. DVE reciprocal costs
                    # ~6cyc per FREE element, so reshape the [1,512] row to
                    # [128,4] via a DRAM bounce, recip, gather back, and
                    # partition-broadcast to [64,512].
                    prow = (h % 2) * HD
                    rdram = dram.tile([1, LC], F32, tag="rdram",
                                      name=f"rd{h}_{qc}")
                    nc.sync.dma_start(out=rdram, in_=ot_sb[HD:HD + 1, :])
                    rsq = btmp.tile([128, LC // 128], F32, tag="rsq",
                                    name=f"rsq{h}_{qc}")
                    nc.sync.dma_start(
                        out=rsq,
                        in_=rdram.rearrange("o (p f) -> (o p) f", p=128))
                    rrec = btmp.tile([128, LC // 128], F32, tag="rrec",
                                     name=f"rrec{h}_{qc}")
                    nc.vector.reciprocal(out=rrec, in_=rsq)
                    rdram2 = dram.tile([1, LC], F32, tag="rdram2",
                                       name=f"rd2{h}_{qc}")
                    nc.sync.dma_start(
                        out=rdram2.rearrange("o (p f) -> (o p) f", p=128),
                        in_=rrec)
                    bc_sb = btmp.tile([HD, LC], F32, tag="bcsb",
                                      name=f"bc{h}_{qc}")
                    bcast_src = bass.AP(
                        tensor=rdram2.tensor, offset=rdram2.offset,
                        ap=[[0, HD]] + list(rdram2.ap[1:]))
                    nc.sync.dma_start(out=bc_sb, in_=bcast_src)
                    nc.vector.tensor_mul(
                        otT[h // 2][prow:prow + HD, qc * LC:(qc + 1) * LC],
                        ot_sb[0:HD, :], bc_sb)

                for hp in range(2):
                    qt = qkT[hp]
                    kt_t = qkT[2 + hp]
                    ha, hb = 2 * hp, 2 * hp + 1
                    vca = slice(ha * (HD + 1), (ha + 1) * (HD + 1))
                    vcb = slice(hb * (HD + 1), (hb + 1) * (HD + 1))
                    for qc in range(NLC):
                        qs = slice(qc * LC, (qc + 1) * LC)
                        ot_a = opsum.tile([HD + 1, LC], F32, tag="otps",
                                          name=f"ota{hp}_{qc}")
                        ot_b = opsum.tile([HD + 1, LC], F32, tag="otps",
                                          name=f"otb{hp}_{qc}")
                        for kt in range(NLT):
                            ks = slice(kt * 128, (kt + 1) * 128)
                            # both heads' S^T into ONE tile: cols 0:512 =
                            # head a, 512:1024 = head b (disjoint row groups)
                            st = spsum.tile([128, 2 * LC], F32, tag="stps",
                                            name=f"st{hp}_{kt}_{qc}")
                            nc.tensor.matmul(
                                st[:, 0:LC],
                                kt_t[0:HD, ks], qt[0:HD, qs],
                                start=True, stop=True,
                                tile_position=(0, 0))
                            nc.tensor.matmul(
                                st[:, LC:2 * LC],
                                kt_t[HD:128, ks], qt[HD:128, qs],
                                start=True, stop=True,
                                tile_position=(HD, 0))
                            e_t = e_pool.tile([128, 2 * LC], F32R, tag="e",
                                              name=f"e{hp}_{kt}_{qc}")
                            nc.scalar.activation(
                                out=e_t, in_=st,
                                func=mybir.ActivationFunctionType.Exp,
                                scale=float(HD) ** -0.5)
                            nc.tensor.matmul(
                                ot_a, v_sb[kt][:, vca], e_t[:, 0:LC],
                                start=(kt == 0), stop=(kt == NLT - 1))
                            nc.tensor.matmul(
                                ot_b, v_sb[kt][:, vcb], e_t[:, LC:2 * LC],
                                start=(kt == 0), stop=(kt == NLT - 1))
                        ota_sb = btmp.tile([HD + 1, LC], F32, tag="otsb",
                                           bufs=4, name=f"otsa{hp}_{qc}")
                        nc.vector.tensor_copy(out=ota_sb, in_=ot_a)
                        otb_sb = btmp.tile([HD + 1, LC], F32, tag="otsb",
                                           bufs=4, name=f"otsb{hp}_{qc}")
                        nc.vector.tensor_copy(out=otb_sb, in_=ot_b)
                        normalize(ota_sb, ha, qc)
                        normalize(otb_sb, hb, qc)

            # ---- phase C: out-projection partial ---------------------------
            with tc.tile_pool(name="ypsum", bufs=4, space="PSUM") as ypsum, \
                 tc.tile_pool(name="ytmp", bufs=4) as ytmp:
                for lt in range(NLT):
                    for nch in range(2):
                        yps = ypsum.tile([128, LC], F32, tag="yps",
                                         name=f"yps{lt}_{nch}")
                        for ft in range(2):
                            nc.tensor.matmul(
                                yps,
                                otT[ft][:, lt * 128:(lt + 1) * 128],
                                wout_sb[ft][:, nch * LC:(nch + 1) * LC],
                                start=(ft == 0), stop=(ft == 1))
                        y_sb = ytmp.tile([128, LC], F32, tag="ysb",
                                         name=f"ysb{lt}_{nch}")
                        if (lt + nch) % 2 == 0:
                            nc.vector.tensor_copy(out=y_sb, in_=yps)
                        else:
                            nc.scalar.copy(out=y_sb, in_=yps)
                        nc.sync.dma_start(
                            out=out_e[lt * 128:(lt + 1) * 128,
                                      nch * LC:(nch + 1) * LC],
                            in_=y_sb)

    nc.compile()
    return nc


def _rope_tables():
    inv_freq = 1.0 / (ROPE_BASE ** (np.arange(0, HD, 2, dtype=np.float32) / HD))
    t = np.arange(L, dtype=np.float32)
    freqs = np.einsum("i,j->ij", t, inv_freq)            # [L, 32]
    emb = np.concatenate((freqs, freqs), axis=-1)        # [L, 64]
    cosT = np.cos(emb).T.astype(np.float32)              # [64, L]
    sinT = np.sin(emb).T.astype(np.float32)              # [64, L]
    # sin table is indexed by the INPUT partition of the rotate_half term:
    # out[0:32] reads in[32:64] -> table rows 32:64 hold -sin;
    # out[32:64] reads in[0:32] -> table rows 0:32 hold +sin.
    cos2 = np.concatenate([cosT, cosT], axis=0)          # [128, L]
    sin_signed = np.concatenate([sinT[:32], -sinT[32:]], axis=0)  # [64, L]
    sin2 = np.concatenate([sin_signed, sin_signed], axis=0)       # [128, L]
    return np.ascontiguousarray(cos2), np.ascontiguousarray(sin2)


_NC = None
TRACE = False          # test harness sets True to collect exec_time_ns
LAST_RESULT = None


def kernel(x, Wqkv, bqkv, Wout, bout):
    global _NC, LAST_RESULT
    if _NC is None:
        _NC = _build_nc()

    x = np.asarray(x, dtype=np.float32)
    Wqkv = np.asarray(Wqkv, dtype=np.float32)
    bqkv = np.asarray(bqkv, dtype=np.float32)
    Wout = np.asarray(Wout, dtype=np.float32)
    bout = np.asarray(bout, dtype=np.float32)

    cos2, sin2 = _rope_tables()

    in_maps = []
    for c in range(N_CORES):
        b = c // 4
        heads = [4 * (c % 4) + i for i in range(HPC)]
        xT = np.ascontiguousarray(x[b].T)                            # [D, L]
        q_cols = [Wqkv[:, h * HD:(h + 1) * HD] for h in heads]
        k_cols = [Wqkv[:, D + h * HD:D + (h + 1) * HD] for h in heads]
        v_cols = [Wqkv[:, 2 * D + h * HD:2 * D + (h + 1) * HD] for h in heads]
        wqk = np.ascontiguousarray(np.concatenate(q_cols + k_cols, axis=1))
        wv = np.ascontiguousarray(np.concatenate(v_cols, axis=1))
        bq = np.concatenate([bqkv[h * HD:(h + 1) * HD] for h in heads])
        bk = np.concatenate([bqkv[D + h * HD:D + (h + 1) * HD] for h in heads])
        bv = np.concatenate([bqkv[2 * D + h * HD:2 * D + (h + 1) * HD]
                             for h in heads])
        wout = np.ascontiguousarray(
            np.concatenate([Wout[h * HD:(h + 1) * HD, :] for h in heads],
                           axis=0))
        in_maps.append({
            "xT": xT.astype(ml_dtypes.bfloat16),
            "wqk": wqk.astype(ml_dtypes.bfloat16),
            "wv": wv.astype(ml_dtypes.bfloat16),
            "wout": wout,
            "cos2": cos2.astype(ml_dtypes.bfloat16),
            "sin2": sin2.astype(ml_dtypes.bfloat16),
            "bqk": np.ascontiguousarray(
                np.concatenate([bq, bk]).reshape(NMT, 128).T),
            "bv": np.ascontiguousarray(bv[None, :]),
            "ones": np.ones((1, LC), dtype=np.float32),
            "vones": np.ones((128, HPC), dtype=np.float32),
        })

    res = run_bass_kernel_spmd(_NC, in_maps, core_ids=list(range(N_CORES)),
                               trace=TRACE)
    LAST_RESULT = res

    out = np.zeros((B, L, D), dtype=np.float32)
    for c in range(N_CORES):
        out[c // 4] += res.results[c]["out"]
    out += bout[None, None, :]
    return out
